# revision 43
# baseline (speedup 1.0000x reference)
"""Multi-head self-attention (B=2, N=2048, D=1024, H=16) on 8 Trainium2 cores.

Sharding: core c -> batch b = c // 4, head group g = c % 4 (heads 4g..4g+3).
Inputs are converted to bf16 on the host (rel-err budget 2e-2 allows it);
each core gets x[b]^T plus its 256-column slices of w_qkv and 256-row slice
of w_proj, and returns one [1024, 2048] bf16 partial-projection tensor.
Host sums 4 partials + residual x per batch.

Per-core design (all matmuls 1.0 cycle/row in the cost model; the two hard
rails are PE ~138us and the 128 ACT exp instructions ~133us):
  load   x^T arrives as one k-split token block then 3 token blocks; q^T/k^T
         for pair 0 start accumulating on the first half; all other q/k
         tiles, v tiles and projections are generator "filler jobs" pumped
         into the main loop at deadline-ordered priorities.
  stream per (pair, i-tile 512, j-chunk 128) - 128 slots:
           scores^T psum [128 j, 2x512 i] (2 matmuls, K=64/head, HI prio)
           exp via one ACT instruction -> e bf16 [128, 1024] (46-deep ring
           so the PV stream can lag the exp stream by an i-tile)
           PV *flipped*: lhsT = e chunk [128 j, 128 i] (free LDWEIGHTS),
           rhs = v|1 [128 j, 65] bf16 -> psum [128 i, 65] accumulated over
           j: only 65 moving rows instead of 512 -> PV costs half.
         normalize: one strided DVE reciprocal of the 8 denominator columns
         + per-partition tensor_scalar muls -> [i, dims] bf16, then
         XBAR-DMA transpose straight into outTn [dims, tok] (no psum, no
         mm-ring coupling).  proj accumulates BOTH pairs into one psum.
  tail   the last tile's normalize uses PE transposes via a borrowed (dead)
         scores-ring tile; its proj evacs alternate ACT/DVE per tile with
         immediate per-tile DMAs.
Priorities: scores chain HI > pv/normalize/v/early-qk HI2 > late pair-1 qk
and proj (normal) -- demoting the late qk tiles lets the pv/normalize chain
claim PE inside the fully-saturated first-pair phase, which is what keeps
the exp stream gapless.
PSUM: sc 2x[128,1024] (4 banks) + pv [128,1024] (2) + mm ring 2x[128,512].
"""

import math
from collections import deque

import numpy as np
import ml_dtypes

import concourse.bass as bass
import concourse.bacc as bacc
import concourse.mybir as mybir
import concourse.tile as tile
from concourse.bass_utils import run_bass_kernel_spmd

B = 2
N = 2048
D = 1024
NH = 16
DH = 64
N_CORES = 8
TP = 4                # head-parallel ways per batch
HPC = NH // TP        # heads per core
HDIM = HPC * DH       # 256 head dims per core
PAIRS = HPC // 2
SCALE = 1.0 / 8.0     # 1/sqrt(DH)

NT = N // 512         # 4 token blocks
KC = D // 128         # 8 feature chunks
JT = N // 128         # 16 j-chunks
IT = N // 512         # 4 i-tiles

BF16 = mybir.dt.bfloat16
F32 = mybir.dt.float32
F32R = mybir.dt.float32r
AF = mybir.ActivationFunctionType

E_BUFS = 46           # deep exp ring decouples ACT from lagging PV
HI = 1_000_000        # high_priority offset for the exp-feeding PE chain
HI2 = 500_000         # pv/normalize tier: above filler, below the sc chain


def build_bass():
    nc = bacc.Bacc("TRN2", target_bir_lowering=False, debug=False)
    xT = nc.declare_dram_parameter("xT", [D, N], BF16, isOutput=False)
    wq = nc.declare_dram_parameter("wq", [D, HDIM], BF16, isOutput=False)
    wk = nc.declare_dram_parameter("wk", [D, HDIM], BF16, isOutput=False)
    wv = nc.declare_dram_parameter("wv", [D, HDIM], BF16, isOutput=False)
    wp = nc.declare_dram_parameter("wp", [HDIM, D], BF16, isOutput=False)
    idd = nc.declare_dram_parameter("idd", [128, 128], BF16, isOutput=False)
    pT = nc.declare_dram_parameter("pT", [D, N], BF16, isOutput=True)

    with tile.TileContext(nc) as tc:
        with (
            tc.tile_pool(name="big", bufs=1) as big,
            tc.tile_pool(name="exps", bufs=1) as exps,
            tc.tile_pool(name="small", bufs=1) as small,
            tc.tile_pool(name="psum", bufs=1, space="PSUM") as psum,
        ):
            idt = big.tile([128, 128], BF16, tag="idt")
            wqs = big.tile([128, KC, HDIM], BF16, tag="wqs")
            wks = big.tile([128, KC, HDIM], BF16, tag="wks")
            wvs = big.tile([128, KC, HDIM], BF16, tag="wvs")
            wps = big.tile([128, PAIRS, D], BF16, tag="wps")
            xts = [big.tile([128, KC, 512], BF16, tag=f"x{nt}", name=f"xts{nt}")
                   for nt in range(NT)]
            qT = big.tile([128, PAIRS, N], F32R, tag="qT")
            kT = big.tile([128, PAIRS, N], F32R, tag="kT")
            v_s = big.tile([128, JT, HPC, 65], BF16, tag="v")
            outTn = big.tile([128, PAIRS, N], BF16, tag="outTn")

            def ld(dst, src):
                nc.sync.dma_start(out=dst, in_=src)

            rearr = lambda ap: ap.rearrange("(k p) c -> p k c", p=128)
            # arrival order tuned: q/k weights + first token block (in two
            # halves so the first q/k matmuls start ~1.5us earlier) first
            warm_src = big.tile([128, 128], BF16, tag="wsrc")
            nc.vector.memset(warm_src, 1.0)
            ld(xts[0][:, 0:4, :],
               xT[0:512, 0:512].rearrange("(k p) c -> p k c", p=128))
            ld(wqs, rearr(wq[:, :]))
            ld(wks, rearr(wk[:, :]))
            ld(xts[0][:, 4:8, :],
               xT[512:1024, 0:512].rearrange("(k p) c -> p k c", p=128))
            ld(idt, idd[:, :])
            ld(xts[1], rearr(xT[:, 512:1024]))
            ld(wvs, rearr(wv[:, :]))
            ld(xts[2], rearr(xT[:, 1024:1536]))
            ld(xts[3], rearr(xT[:, 1536:2048]))
            ld(wps, wp[:, :].rearrange("(r p) c -> p r c", p=128))
            nc.vector.memset(v_s[:, :, :, 64:65], 1.0)

            def warm_pe(n):
                # keep the PE p-state ramp warm while DMAs land
                for _ in range(n):
                    t = psum.tile([128, 512], F32, tag="mm", bufs=2)
                    nc.tensor.transpose(t[:, 0:64].bitcast(BF16), warm_src,
                                        warm_src)

            # ---- filler jobs (generators; one yield ~= 400-900ns of PE) ----
            def qk_job(p, w_sb, dst, nt, hi=True):
                ps = psum.tile([128, 512], F32, tag="mm", bufs=2)
                for k in range(KC):
                    with tc.high_priority(HI2 if hi else 0):
                        nc.tensor.matmul(
                            ps,
                            lhsT=w_sb[:, k, p * 128:(p + 1) * 128],
                            rhs=xts[nt][:, k, :],
                            start=(k == 0),
                            stop=(k == KC - 1),
                        )
                    if k % 2 == 1:
                        yield
                with tc.high_priority(HI):
                    nc.vector.tensor_copy(dst[:, p, nt * 512:(nt + 1) * 512], ps)
                yield

            v_emitted = 0

            def v_job(jt):
                nonlocal v_emitted
                nt, tc4 = divmod(jt, 4)
                ps = psum.tile([128, 512], F32, tag="mm", bufs=2)
                for k in range(KC):
                    with tc.high_priority(HI2):
                        nc.tensor.matmul(
                            ps[:, 0:HDIM],
                            lhsT=xts[nt][:, k, tc4 * 128:(tc4 + 1) * 128],
                            rhs=wvs[:, k, :],
                            start=(k == 0),
                            stop=(k == KC - 1),
                        )
                    if k % 4 == 3:
                        yield
                with tc.high_priority(HI2):
                    nc.vector.tensor_copy(
                        v_s[:, jt, :, 0:64],
                        ps[:, 0:HDIM].rearrange("p (h c) -> p h c", c=64),
                    )
                v_emitted += 1
                yield

            # last-tile proj: evacs alternate between the (idle-by-then) ACT
            # engine and DVE, DMA'd out in pair-batched transfers.
            ob2_ring = []

            def proj_job(ot, tt, tail=False):
                pj = psum.tile([128, 512], F32, tag="mm", bufs=2)
                for p in range(PAIRS):
                    nc.tensor.matmul(
                        pj,
                        lhsT=wps[:, p, ot * 128:(ot + 1) * 128],
                        rhs=outTn[:, p, tt * 512:(tt + 1) * 512],
                        start=(p == 0),
                        stop=(p == PAIRS - 1),
                    )
                yield
                if tail:
                    ob = small.tile([128, 512], BF16, tag="ob", bufs=4)
                    if ot % 2 == 0:
                        nc.scalar.copy(ob, pj)
                    else:
                        nc.vector.tensor_copy(ob, pj)
                    nc.sync.dma_start(
                        out=pT[ot * 128:(ot + 1) * 128, tt * 512:(tt + 1) * 512],
                        in_=ob,
                    )
                else:
                    ob = small.tile([128, 512], BF16, tag="ob", bufs=4)
                    nc.vector.tensor_copy(ob, pj)
                    nc.sync.dma_start(
                        out=pT[ot * 128:(ot + 1) * 128, tt * 512:(tt + 1) * 512],
                        in_=ob,
                    )
                yield

            # (deadline_slot, generator): deadline = slot whose sc READS the
            # tensor this job writes; emission must precede that slot.
            # Deadlines are non-decreasing (pump only force-drains the head).
            jobs = deque()
            jobs.append((4, qk_job(0, wks, kT, 1)))
            jobs.append((8, qk_job(0, wks, kT, 2)))
            jobs.append((12, qk_job(0, wks, kT, 3)))
            jobs.append((14, qk_job(0, wqs, qT, 1)))
            for jt in range(4):
                jobs.append((16 + jt, v_job(jt)))
            jobs.append((20, qk_job(0, wqs, qT, 2)))
            for jt in range(4, 10):
                jobs.append((18 + jt, v_job(jt)))
            jobs.append((28, qk_job(0, wqs, qT, 3)))
            for jt in range(10, 16):
                jobs.append((20 + jt, v_job(jt)))
            jobs.append((44, qk_job(1, wks, kT, 0)))
            jobs.append((46, qk_job(1, wqs, qT, 0)))
            jobs.append((48, qk_job(1, wks, kT, 1)))
            jobs.append((52, qk_job(1, wks, kT, 2, hi=False)))
            jobs.append((56, qk_job(1, wks, kT, 3, hi=False)))
            jobs.append((60, qk_job(1, wqs, qT, 1)))
            jobs.append((70, qk_job(1, wqs, qT, 2, hi=False)))
            jobs.append((84, qk_job(1, wqs, qT, 3, hi=False)))

            def pump(slot, steps=2):
                while jobs and jobs[0][0] <= slot:
                    for _ in jobs.popleft()[1]:
                        pass
                for _ in range(steps):
                    if not jobs:
                        return
                    try:
                        next(jobs[0][1])
                    except StopIteration:
                        jobs.popleft()

            # ---- pv-phase stream (ordered; gated on v emission) ----
            pvq = deque()

            def emit_pv(p, it, jt, e_t, pv_t):
                pri = HI if (p == 1 and it == IT - 1) else HI2
                with tc.high_priority(pri):
                    for h in range(2):
                        for ic in range(4):
                            g = h * 4 + ic
                            nc.tensor.matmul(
                                pv_t[:, g * 128:g * 128 + 65],
                                lhsT=e_t[:, h * 512 + ic * 128:h * 512 + (ic + 1) * 128],
                                rhs=v_s[:, jt, 2 * p + h, :],
                                start=(jt == 0),
                                stop=(jt == JT - 1),
                            )

            def emit_fin(p, it, pv_t):
                tail = (p == 1 and it == IT - 1)
                rc = small.tile([128, 8], F32, tag="rc", bufs=2)
                pvr = pv_t.rearrange("p (g c) -> p g c", c=128)
                xp_t = None
                if tail:
                    # the scores ring is dead at the tail: borrow one tile as
                    # low-latency PE-transpose scratch (the XBAR-DMA path's
                    # ~3us latency would gate the final proj otherwise)
                    xp_t = psum.tile([128, 1024], F32, tag="sc", bufs=2,
                                     name="xp_t")
                nss = []
                with tc.high_priority(HI2):
                    nc.vector.reciprocal(
                        rc, pvr[:, :, 64:65].rearrange("p g c -> p (g c)")
                    )
                    # staged per-group muls: byte-granular pv reads release the
                    # pv ring group-by-group for the next tile's accumulation
                    for ic in range(4):
                        ns = small.tile([128, 128], BF16, tag="ns", bufs=8)
                        nss.append(ns)
                        for h in range(2):
                            g = h * 4 + ic
                            dst = ns[:, h * 64:(h + 1) * 64]
                            if tail and h == 1:
                                nc.scalar.activation(
                                    dst, pv_t[:, g * 128:g * 128 + 64],
                                    AF.Copy, scale=rc[:, g:g + 1],
                                )
                            else:
                                nc.vector.tensor_scalar_mul(
                                    dst, pv_t[:, g * 128:g * 128 + 64],
                                    rc[:, g:g + 1],
                                )
                    for ic in range(4):
                        ns = nss[ic]
                        if tail:
                            xpv = xp_t[:, ic * 256:ic * 256 + 64].bitcast(BF16)
                            nc.tensor.transpose(xpv, ns, idt)
                            nc.vector.tensor_copy(
                                outTn[:, p,
                                      it * 512 + ic * 128:it * 512 + (ic + 1) * 128],
                                xpv,
                            )
                        else:
                            # XBAR transpose straight into outTn: no psum
                            # tile, no DVE copy, no mm-ring coupling
                            nc.sync.dma_start(
                                out=outTn[:, p,
                                          it * 512 + ic * 128:it * 512 + (ic + 1) * 128],
                                in_=ns,
                                transpose=True,
                            )
                if p == 1:
                    for ot in range(D // 128):
                        jobs.append((math.inf, proj_job(ot, it, tail=tail)))

            def drain_pvq():
                while pvq:
                    item = pvq[0]
                    if item[0] == "pv":
                        _, p, it, jt, e_t, pv_t = item
                        if jt >= v_emitted:
                            return
                        emit_pv(p, it, jt, e_t, pv_t)
                    else:
                        _, p, it, pv_t = item
                        emit_fin(p, it, pv_t)
                    pvq.popleft()

            # ---- prologue PE work ----
            warm_pe(18)

            # nt0 split by contraction-half so accumulation starts on the
            # first k-half DMA while the second still transfers
            qk0_ps = {}
            for w_sb, key in ((wqs, "q"), (wks, "k")):
                ps = psum.tile([128, 512], F32, tag="mm", bufs=2, name="qk0")
                qk0_ps[key] = ps
                for k in range(4):
                    nc.tensor.matmul(
                        ps, lhsT=w_sb[:, k, 0:128], rhs=xts[0][:, k, :],
                        start=(k == 0), stop=False,
                    )
            for w_sb, key, dst in ((wqs, "q", qT), (wks, "k", kT)):
                ps = qk0_ps[key]
                for k in range(4, KC):
                    nc.tensor.matmul(
                        ps, lhsT=w_sb[:, k, 0:128], rhs=xts[0][:, k, :],
                        start=False, stop=(k == KC - 1),
                    )
                with tc.high_priority(HI):
                    if key == "k":
                        nc.scalar.copy(dst[:, 0, 0:512], ps)
                    else:
                        nc.vector.tensor_copy(dst[:, 0, 0:512], ps)

            # ---- main loop ----
            slot = 0
            for p in range(PAIRS):
                for it in range(IT):
                    pv_t = psum.tile([128, 1024], F32, tag="pv", bufs=1)
                    for jt in range(JT):
                        pump(slot, steps=3 if slot < 56 else 2)
                        sc = psum.tile([128, 1024], F32, tag="sc", bufs=2)
                        with tc.high_priority(HI):
                            for h in range(2):
                                nc.tensor.matmul(
                                    sc[:, h * 512:(h + 1) * 512],
                                    lhsT=kT[h * 64:(h + 1) * 64, p,
                                            jt * 128:(jt + 1) * 128],
                                    rhs=qT[h * 64:(h + 1) * 64, p,
                                           it * 512:(it + 1) * 512],
                                    start=True,
                                    stop=True,
                                )
                        e_t = exps.tile([128, 1024], BF16, tag="e", bufs=E_BUFS)
                        nc.scalar.activation(e_t, sc, AF.Exp, scale=SCALE)
                        pvq.append(("pv", p, it, jt, e_t, pv_t))
                        if jt == JT - 1:
                            pvq.append(("fin", p, it, pv_t))
                        drain_pvq()
                        slot += 1

            # ---- drain ----
            while jobs or pvq:
                pump(slot, steps=4)
                drain_pvq()
                slot += 1
                if slot > 1000:
                    raise RuntimeError("emission drain did not converge")
    return nc


_NC = None


def _get_nc():
    global _NC
    if _NC is None:
        _NC = build_bass()
        _NC.finalize()
    return _NC


_IDENT = np.eye(128, dtype=ml_dtypes.bfloat16)


def make_in_maps(x, w_qkv, w_proj):
    bf = ml_dtypes.bfloat16
    x = np.asarray(x, np.float32)
    w_qkv = np.asarray(w_qkv, np.float32)
    w_proj = np.asarray(w_proj, np.float32)
    xTs = [np.ascontiguousarray(x[b].T).astype(bf) for b in range(B)]
    in_maps = []
    for c in range(N_CORES):
        b, g = divmod(c, TP)
        h0 = g * HDIM
        in_maps.append({
            "xT": xTs[b],
            "wq": np.ascontiguousarray(w_qkv[:, h0:h0 + HDIM]).astype(bf),
            "wk": np.ascontiguousarray(w_qkv[:, D + h0:D + h0 + HDIM]).astype(bf),
            "wv": np.ascontiguousarray(w_qkv[:, 2 * D + h0:2 * D + h0 + HDIM]).astype(bf),
            "wp": np.ascontiguousarray(w_proj[h0:h0 + HDIM, :]).astype(bf),
            "idd": _IDENT,
        })
    return in_maps


def combine_outputs(x, results):
    x = np.asarray(x, np.float32)
    out = np.empty((B, N, D), np.float32)
    for b in range(B):
        acc = x[b].astype(np.float64)
        for g in range(TP):
            acc += results[b * TP + g]["pT"].T.astype(np.float32)
        out[b] = acc.astype(np.float32)
    return out


def kernel(x, w_qkv, w_proj):
    nc = _get_nc()
    in_maps = make_in_maps(x, w_qkv, w_proj)
    res = run_bass_kernel_spmd(nc, in_maps, list(range(N_CORES))).results
    return combine_outputs(x, res)
